# revision 35
# baseline (speedup 1.0000x reference)
"""GATv2 8-hop GNN on 8 Trainium2 NeuronCores (Bass/Tile).

Strategy (receiver-sharded, degree-tiered, quad-gather):
  - Nodes are partitioned across the 8 cores balanced by in-degree; each core
    owns its nodes' full in-edge lists (edges sharded by receiver).
  - Per core, nodes are grouped into degree tiers (padded to d in TIERS) and
    packed into tiles of 128 nodes x d edge slots.  Edge data / sender
    indices / masks are laid out tile-major on the host (numpy) so the
    device kernel is a uniform For_i loop per tier.
  - Per hop: each core computes kn|q = h @ [Wk|Wq] for its own nodes (one
    PE matmul per tile from a PE-transposed h), writes kn (f16) to DRAM,
    AllGathers the node-indexed kn table, then processes its tiles:
      * ONE dma_gather per tile fetches, for every edge slot, the 512-byte
        quad-row of the sender (4 nodes' kn packed per row; the table is
        just kn_full viewed as [N/4, 256]).  int16 quad indices fit because
        N/4 < 32768.  Gathers spread over 4 SWDGE queues.
      * a 4-way one-hot select on DVE (mult by host-built sel mask, then
        reduce over the quad axis) extracts each slot's sender row kn[snd].
      * kq = kn[snd] + qb broadcast; per chunk of 8 slots: z = e@We + kq
        (PE matmuls for e@We into PSUM, one DVE add), leaky-relu via
        u = z + (2/3)|z| (logits scaled by 0.6a), exp, mask, and
        mex-weighted reductions on DVE produce den and pooled.
      * h' = relu((pooled/den - qb) * valid)
  - Output: per-core column sums of h; host sums across cores, divides by N.
"""
import os as _os
import sys

sys.path.insert(0, "/opt/trn_rl_repo")

import numpy as np
from contextlib import ExitStack

import concourse.bass as bass
import concourse.mybir as mybir
import concourse.tile as tile
from concourse.bass import AP, ds
from concourse.masks import make_identity

P = 128
H = 64
HOPS = int(_os.environ.get('GAT_HOPS', '8'))
NCORES = 8
CH = 8  # slots per PSUM chunk
NQ = 4  # SWDGE queues for gathers
TIERS = [8, 16, 24, 32, 48, 64, 96, 128]
EPS = 1e-30
F16 = mybir.dt.float16
F32 = mybir.dt.float32
I16 = mybir.dt.int16
U8 = mybir.dt.uint8


def make_bacc(bacc_mod, num_devices=NCORES):
    return bacc_mod.Bacc("TRN2", target_bir_lowering=False, debug=False,
                         num_devices=num_devices, num_swdge_queues=NQ)


# ----------------------------------------------------------------- host prep
def preprocess(node_feats, edge_feats, senders, receivers, Wq, Wk, We, b, att):
    """Shard + tile the graph.  Returns (meta, in_maps)."""
    N = node_feats.shape[0]
    E = edge_feats.shape[0]
    deg = np.bincount(receivers, minlength=N)
    assert deg.max() <= TIERS[-1], f"max degree {deg.max()} > {TIERS[-1]}"

    cum = np.cumsum(deg)
    bounds = [0] + [int(np.searchsorted(cum, E * (c + 1) / NCORES))
                    for c in range(NCORES - 1)] + [N]

    tier_arr = np.asarray(TIERS)
    tier_of = np.searchsorted(tier_arr, np.maximum(deg, 1))
    core_tier_nodes = []
    for c in range(NCORES):
        lo, hi = bounds[c], bounds[c + 1]
        nodes = np.arange(lo, hi)
        t_of = tier_of[lo:hi]
        core_tier_nodes.append([nodes[t_of == ti] for ti in range(len(TIERS))])

    T_tier = [max(int(np.ceil(len(core_tier_nodes[c][ti]) / P))
                  for c in range(NCORES)) for ti in range(len(TIERS))]
    used = [ti for ti in range(len(TIERS)) if T_tier[ti] > 0]
    tiers_d = [TIERS[ti] for ti in used]
    tiers_T = [T_tier[ti] for ti in used]
    Ttot = sum(tiers_T)
    n_slab = Ttot * P
    assert (NCORES * n_slab) % 4 == 0
    assert (NCORES * n_slab) // 4 < 32768, "quad index must fit int16"

    # Global table position: kn_full is laid out [ag_chunk, core, half_slab]
    # so the AllGather can run as two half-table collectives overlapped with
    # phase A.  half must be a multiple of 4 for quad indexing.
    half = (n_slab // 2 + P - 1) // P * P
    assert half % 4 == 0

    def pos_of(c, o):
        ch = o // half
        return ch * NCORES * half + c * half + (o - ch * half)

    pos = np.zeros(N, np.int64)
    slab_node = np.full((NCORES, n_slab), -1, np.int64)
    for c in range(NCORES):
        off = 0
        for j, ti in enumerate(used):
            nodes = core_tier_nodes[c][ti]
            pos[nodes] = pos_of(c, off + np.arange(len(nodes)))
            slab_node[c, off:off + len(nodes)] = nodes
            off += tiers_T[j] * P
    assert off == n_slab

    order = np.argsort(receivers, kind="stable")
    starts = np.zeros(N + 1, np.int64)
    np.cumsum(deg, out=starts[1:])

    ef16 = np.concatenate([np.asarray(edge_feats).astype(np.float16),
                           np.zeros((1, H), np.float16)], axis=0)
    snd_pos = pos[senders]

    in_maps = []
    for c in range(NCORES):
        m = {}
        off = 0
        for j, ti in enumerate(used):
            d = tiers_d[j]
            Tt = tiers_T[j]
            rows = slab_node[c, off:off + Tt * P]
            eids = np.full((Tt * P, d), -1, np.int64)
            real = rows >= 0
            idxr = np.nonzero(real)[0]
            if len(idxr):
                rs = starts[rows[idxr]]
                dd = deg[rows[idxr]]
                ar = np.arange(d)
                sel = ar[None, :] < dd[:, None]
                flat = rs[:, None] + ar[None, :]
                tmp = np.full((len(idxr), d), -1, np.int64)
                tmp[sel] = order[flat[sel]]
                eids[idxr] = tmp
            pad = eids < 0
            e_safe = np.where(pad, E, eids)
            slab = ef16[e_safe].reshape(Tt, P, d, H).transpose(0, 2, 3, 1).copy()
            m[f"edge{j}"] = slab
            spos = np.where(pad, 0, snd_pos[np.maximum(eids, 0)])
            qid = (spos >> 2).astype(np.int16)          # [Tt*P, d]
            phase = (spos & 3).astype(np.int64)
            # int16 wrapped layout for dma_gather: flat i = s*128 + p sits at
            # [i % 16, i // 16], replicated over the 8 gpsimd cores (128 rows).
            qid_t = qid.reshape(Tt, P, d)
            arr = qid_t.transpose(0, 2, 1).reshape(Tt, d * P)   # flat s*128+p
            w16 = arr.reshape(Tt, d * P // 16, 16).transpose(0, 2, 1)
            m[f"idx{j}"] = np.tile(w16, (1, 8, 1)).copy()       # [Tt,128,8d]
            # copy_predicated planes for quad phases 1..3 (phase 0 = base copy)
            mp = np.zeros((Tt * P, d, 3), np.uint8)
            pr = np.nonzero(~pad)
            ph = phase[~pad]
            hit = ph >= 1
            mp[pr[0][hit], pr[1][hit], ph[hit] - 1] = 1.0
            m[f"sel{j}"] = mp.reshape(Tt, P, d, 3).copy()
            m[f"mask{j}"] = (~pad).astype(np.float16).reshape(Tt, P, d).copy()
            off += Tt * P
        rows = slab_node[c]
        h0 = np.zeros((n_slab, H), np.float32)
        rr = rows >= 0
        h0[rr] = np.asarray(node_feats)[rows[rr]].astype(np.float32)
        m["h0"] = h0.reshape(Ttot, P, H).copy()
        m["valid"] = rr.astype(np.float32).reshape(Ttot, P).T.copy()
        wkq = np.concatenate([np.asarray(Wk)[:HOPS], np.asarray(Wq)[:HOPS]],
                             axis=2).astype(np.float16)         # [HOPS,64,128]
        m["wkq"] = wkq
        m["we"] = np.asarray(We)[:HOPS].astype(np.float16)
        m["brep"] = np.broadcast_to(
            np.asarray(b)[:HOPS].astype(np.float16)[:, None, :], (HOPS, P, H)).copy()
        m["arep"] = np.broadcast_to(
            np.asarray(att)[:HOPS].astype(np.float16)[:, None, :],
            (HOPS, P, H)).copy()
        in_maps.append(m)

    meta = dict(tiers_d=tiers_d, tiers_T=tiers_T, Ttot=Ttot, n_slab=n_slab,
                slab_node=slab_node, half=half)
    return meta, in_maps


# -------------------------------------------------------------- device build
def build_program(tc, t_in, t_out, meta):
    """Emit the program into TileContext tc.  t_in: dict name->AP (DRAM)."""
    nc = tc.nc
    tiers_d = meta["tiers_d"]
    tiers_T = meta["tiers_T"]
    Ttot = meta["Ttot"]
    n_slab = meta["n_slab"]

    kn_own = nc.dram_tensor("kn_own", [n_slab, H], F16, kind="Internal")
    kn_full = nc.dram_tensor("kn_full", [NCORES * n_slab, H], F16,
                             kind="Internal", addr_space="Shared")
    kn_own_t = kn_own.ap().rearrange("(t p) f -> t p f", p=P)
    kn_quad = kn_full.ap().rearrange("(q x) f -> q (x f)", x=4)

    SKIP_A = _os.environ.get("GAT_SKIP_A")
    SKIP_AG = _os.environ.get("GAT_SKIP_AG")
    SKIP_C = _os.environ.get("GAT_SKIP_C")
    SKIP_GATHER = _os.environ.get("GAT_SKIP_GATHER")

    with ExitStack() as ctx:
        const = ctx.enter_context(tc.tile_pool(name="const", bufs=1))

        id32 = const.tile([P, P], F32)
        make_identity(nc, id32[:])
        id16 = const.tile([P, P], F16)
        make_identity(nc, id16[:])

        wkq_sb = const.tile([H, HOPS, 2 * H], F16)
        nc.sync.dma_start(wkq_sb[:], t_in["wkq"].rearrange("i k f -> k i f"))
        we_sb = const.tile([H, HOPS, H], F16)
        nc.sync.dma_start(we_sb[:], t_in["we"].rearrange("i k f -> k i f"))
        brep_sb = const.tile([P, HOPS, H], F16)
        nc.sync.dma_start(brep_sb[:], t_in["brep"].rearrange("i p f -> p i f"))
        arep_sb = const.tile([P, HOPS, H], F16)
        nc.sync.dma_start(arep_sb[:], t_in["arep"].rearrange("i p f -> p i f"))
        valid_sb = const.tile([P, Ttot], F32)
        nc.sync.dma_start(valid_sb[:], t_in["valid"])

        h_own = const.tile([P, Ttot, H], F32)
        nc.sync.dma_start(h_own[:], t_in["h0"].rearrange("t p f -> p t f"))
        qb_own = const.tile([P, Ttot, H], F16)

        # hop-invariant per-tile constants kept resident in SBUF
        sel_c = {}
        mask_c = {}
        idx_c = {}
        for j, d in enumerate(tiers_d):
            Tt = tiers_T[j]
            sel_cj = const.tile([P, Tt, d, 3], U8, tag=f"sel_c{j}")
            nc.sync.dma_start(sel_cj[:], t_in[f"sel{j}"].rearrange(
                "t p d q -> p t d q"))
            mask_cj = const.tile([P, Tt, d], F16, tag=f"mask_c{j}")
            nc.sync.dma_start(mask_cj[:], t_in[f"mask{j}"].rearrange(
                "t p d -> p t d"))
            idx_cj = const.tile([P, Tt, 8 * d], I16, tag=f"idx_c{j}")
            nc.sync.dma_start(idx_cj[:], t_in[f"idx{j}"].rearrange(
                "t p c -> p t c"))
            sel_c[j] = sel_cj
            mask_c[j] = mask_cj
            idx_c[j] = idx_cj

        half = meta["half"]
        T_half = half // P

        def emit_ag(lo_rows, hi_rows, out_lo):
            nc.gpsimd.collective_compute(
                "AllGather", mybir.AluOpType.bypass,
                replica_groups=[list(range(NCORES))],
                ins=[kn_own.ap()[lo_rows:hi_rows]],
                outs=[kn_full.ap()[out_lo:out_lo
                                   + NCORES * (hi_rows - lo_rows)]])

        for hop in range(HOPS):
            # ---- phase A: kn table (DRAM) + qb (SBUF) for own nodes.
            # Fully unrolled; the AllGather runs as two half-table chunks,
            # the first overlapping with the second half of phase A.
            with tc.tile_pool(name=f"psA{hop}", bufs=4, space="PSUM") as psA, \
                 tc.tile_pool(name=f"sbA{hop}", bufs=8) as sbA:
                def bodyA(t):
                    h_t = h_own[:, ds(t, 1), :].rearrange("p o f -> p (o f)")
                    h_stage = sbA.tile([P, H], F32, tag="h_stage")
                    nc.vector.tensor_copy(h_stage[:], h_t)
                    trp = psA.tile([H, P], F32, tag="trp")
                    nc.tensor.transpose(trp[:], h_stage[:], id32[:])
                    hT16 = sbA.tile([H, P], F16, tag="hT16")
                    nc.vector.tensor_copy(hT16[:], trp[:])
                    knq = psA.tile([P, 2 * H], F32, tag="knq")
                    nc.tensor.matmul(knq[:], lhsT=hT16[:],
                                     rhs=wkq_sb[:, hop, :], start=True, stop=True)
                    kn16 = sbA.tile([P, H], F16, tag="kn16")
                    nc.scalar.copy(kn16[:], knq[:, 0:H])
                    nc.sync.dma_start(
                        kn_own_t[ds(t, 1)].rearrange("o p f -> p (o f)"), kn16[:])
                    qb_t = qb_own[:, ds(t, 1), :].rearrange("p o f -> p (o f)")
                    nc.vector.tensor_tensor(out=qb_t, in0=knq[:, H:2 * H],
                                            in1=brep_sb[:, hop, :],
                                            op=mybir.AluOpType.add)

                if not SKIP_A:
                    for t in range(T_half):
                        bodyA(t)
                    if not SKIP_AG:
                        emit_ag(0, half, 0)
                    for t in range(T_half, Ttot):
                        bodyA(t)
                    if not SKIP_AG:
                        emit_ag(half, n_slab, NCORES * half)
                else:
                    nc.vector.memset(qb_own[:], 0)
                    if not SKIP_AG:
                        emit_ag(0, half, 0)
                        emit_ag(half, n_slab, NCORES * half)

            # ---- phase C: edge processing, one For_i per tier
            tile_base = 0
            for j, d in enumerate(tiers_d):
                Tt = tiers_T[j]
                nchunk = (d + CH - 1) // CH
                qb_tier = qb_own[:, tile_base:tile_base + Tt, :]
                h_tier = h_own[:, tile_base:tile_base + Tt, :]
                valid_tier = valid_sb[:, tile_base:tile_base + Tt]
                gbufs = 4 if d <= 24 else 2
                with tc.tile_pool(name=f"psC{hop}_{j}", bufs=4, space="PSUM") as psC, \
                     tc.tile_pool(name=f"psP{hop}_{j}", bufs=4, space="PSUM") as psP, \
                     tc.tile_pool(name=f"gth{hop}_{j}", bufs=gbufs) as gth, \
                     tc.tile_pool(name=f"sbC{hop}_{j}", bufs=4 if d <= 24 else 2) as sbC:
                    def bodyC(t, qnum, d=d, j=j, Tt=Tt, nchunk=nchunk,
                              qb_tier=qb_tier, h_tier=h_tier,
                              valid_tier=valid_tier, psC=psC, psP=psP,
                              gth=gth, sbC=sbC):
                        knr4 = gth.tile([P, d, 4 * H], F16, tag="knr4")
                        if SKIP_GATHER:
                            nc.vector.memset(knr4[:], 0)
                        else:
                            nc.gpsimd.dma_gather(
                                out_ap=knr4[:, :, :],
                                in_ap=kn_quad,
                                idxs_ap=idx_c[j][:, ds(t, 1), :].rearrange(
                                    "p o c -> p (o c)"),
                                num_idxs=d * P,
                                num_idxs_reg=d * P,
                                elem_size=4 * H,
                                single_packet=False,
                                queue_num=qnum)
        # same math as reference: leaky_relu(z, 0.2) = max(0.2*z, z)
                        qb_t = qb_tier[:, ds(t, 1), :].rearrange("p o f -> p (o f)")
                        mask_t = mask_c[j][:, ds(t, 1), :].rearrange(
                            "p o d -> p (o d)")
                        z16all = sbC.tile([P, d, H], F16, tag="z16all")
                        lgall = sbC.tile([P, d], F32, tag="lgall")
                        mexall = sbC.tile([P, d], F16, tag="mexall")
                        pooled_ps = psP.tile([P, H], F32, tag="pool")
                        for k in range(nchunk):
                            c0 = k * CH
                            cw = min(CH, d - c0)
                            # quad select: copy phase-0 candidate, then
                            # overwrite with phases 1..3 via copy_predicated
                            kq = sbC.tile([P, CH, H], F16, tag="kq")
                            nc.vector.tensor_copy(
                                kq[:, 0:cw, :], knr4[:, c0:c0 + cw, 0:H])
                            for q in range(1, 4):
                                _s = sel_c[j][:, ds(t, 1), c0:c0 + cw,
                                              q - 1:q]
                                mp_b = AP(_s.tensor, _s.offset,
                                          [list(_s.ap[0]), list(_s.ap[2]),
                                           [0, H]])
                                nc.vector.copy_predicated(
                                    kq[:, 0:cw, :], mp_b,
                                    knr4[:, c0:c0 + cw, q * H:(q + 1) * H])
                            edge_sb = sbC.tile([H, CH, P], F16, tag="edge")
                            nc.sync.dma_start(
                                edge_sb[:, 0:cw, :],
                                t_in[f"edge{j}"][ds(t, 1), c0:c0 + cw]
                                .rearrange("o s f p -> f (o s) p"))
                            zps = psC.tile([P, CH * H], F32, tag="z")
                            for s in range(cw):
                                sl = slice(s * H, (s + 1) * H)
                                nc.tensor.matmul(zps[:, sl], lhsT=edge_sb[:, s, :],
                                                 rhs=we_sb[:, hop, :],
                                                 start=True, stop=True)
                            _q = qb_t
                            qb_b = AP(_q.tensor, _q.offset,
                                      [list(_q.ap[0]), [0, cw], list(_q.ap[1])])
                            kqq = sbC.tile([P, CH, H], F16, tag="kqq")
                            nc.vector.tensor_tensor(out=kqq[:, 0:cw, :],
                                                    in0=kq[:, 0:cw, :],
                                                    in1=qb_b,
                                                    op=mybir.AluOpType.add)
                            z16c = z16all[:, c0:c0 + cw, :]
                            nc.vector.tensor_tensor(
                                out=z16c,
                                in0=zps[:, 0:cw * H].rearrange(
                                    "p (c h) -> p c h", h=H),
                                in1=kqq[:, 0:cw, :],
                                op=mybir.AluOpType.add)
                            # leaky_relu(z, 0.2) = max(0.2*z, z); u/au/logits
                            # run on the (otherwise idle) gpsimd vector units
                            u = sbC.tile([P, CH, H], F16, tag="u")
                            nc.vector.scalar_tensor_tensor(
                                out=u[:, 0:cw, :], in0=z16c,
                                scalar=0.2, in1=z16c,
                                op0=mybir.AluOpType.mult, op1=mybir.AluOpType.max)
                            au = sbC.tile([P, CH, H], F16, tag="au")
                            _a = arep_sb[:, hop, :]
                            a_b = AP(_a.tensor, _a.offset,
                                     [list(_a.ap[0]), [0, cw], list(_a.ap[1])])
                            nc.gpsimd.tensor_tensor(
                                out=au[:, 0:cw, :],
                                in0=u[:, 0:cw, :],
                                in1=a_b, op=mybir.AluOpType.mult)
                            nc.vector.tensor_reduce(
                                out=lgall[:, c0:c0 + cw], in_=au[:, 0:cw, :],
                                axis=mybir.AxisListType.X, op=mybir.AluOpType.add)
                        # tile-wide softmax pieces: one Exp per tile keeps the
                        # ACT engine on a single function table
                        ex = sbC.tile([P, d], F32, tag="ex")
                        nc.scalar.activation(ex[:], lgall[:],
                                             mybir.ActivationFunctionType.Exp)
                        nc.vector.tensor_tensor(
                            out=mexall[:], in0=ex[:], in1=mask_t,
                            op=mybir.AluOpType.mult)
                        for k in range(nchunk):
                            c0 = k * CH
                            cw = min(CH, d - c0)
                            exz = sbC.tile([P, CH, H], F16, tag="exz")
                            _m = mexall[:, c0:c0 + cw]
                            m_b = AP(_m.tensor, _m.offset,
                                     [list(_m.ap[0]), list(_m.ap[1]), [0, H]])
                            nc.vector.tensor_tensor(
                                out=exz[:, 0:cw, :],
                                in0=z16all[:, c0:c0 + cw, :],
                                in1=m_b, op=mybir.AluOpType.mult)
                            # pooled += sum_s exz_s on the idle PE via
                            # identity matmuls (avoids slow strided reduces)
                            for s in range(cw):
                                nc.tensor.matmul(
                                    pooled_ps[:], lhsT=id16[:],
                                    rhs=exz[:, s, :],
                                    start=(k == 0 and s == 0),
                                    stop=(k == nchunk - 1 and s == cw - 1))
                        den_r = sbC.tile([P, 1], F32, tag="den_r")
                        nc.vector.tensor_reduce(
                            out=den_r[:], in_=mexall[:],
                            axis=mybir.AxisListType.X, op=mybir.AluOpType.add)
                        den = sbC.tile([P, 1], F32, tag="den")
                        nc.vector.tensor_scalar(
                            out=den[:], in0=den_r[:], scalar1=EPS,
                            scalar2=None, op0=mybir.AluOpType.add)
                        rden = sbC.tile([P, 1], F32, tag="rden")
                        nc.vector.reciprocal(rden[:], den[:])
                        t1 = sbC.tile([P, H], F32, tag="t1")
                        nc.vector.scalar_tensor_tensor(
                            out=t1[:], in0=pooled_ps[:], scalar=rden[:],
                            in1=qb_t,
                            op0=mybir.AluOpType.mult,
                            op1=mybir.AluOpType.subtract)
                        h_t = h_tier[:, ds(t, 1), :].rearrange("p o f -> p (o f)")
                        nc.vector.tensor_scalar(
                            out=h_t, in0=t1[:], scalar1=valid_tier[:, ds(t, 1)],
                            scalar2=0.0, op0=mybir.AluOpType.mult,
                            op1=mybir.AluOpType.max)

                    if not SKIP_C:
                        for t in range(Tt):
                            bodyC(t, (tile_base + t) % NQ)
                tile_base += Tt

        hsum = const.tile([P, H], F32)
        nc.vector.tensor_reduce(out=hsum[:],
                                in_=h_own[:].rearrange("p t f -> p f t"),
                                axis=mybir.AxisListType.X, op=mybir.AluOpType.add)
        nc.sync.dma_start(t_out, hsum[:])


# --------------------------------------------------------------------- entry
def kernel(node_feats, edge_feats, senders, receivers, Wq, Wk, We, b, att):
    from concourse import bacc
    from concourse.bass_utils import run_bass_kernel_spmd

    node_feats = np.asarray(node_feats)
    meta, in_maps = preprocess(node_feats, edge_feats, senders, receivers,
                               Wq, Wk, We, b, att)
    nc = make_bacc(bacc)
    t_in = {}
    for k, v in in_maps[0].items():
        t_in[k] = nc.dram_tensor(k, list(v.shape), mybir.dt.from_np(v.dtype),
                                 kind="ExternalInput").ap()
    t_out = nc.dram_tensor("out", [P, H], F32, kind="ExternalOutput").ap()
    with tile.TileContext(nc) as tc:
        build_program(tc, t_in, t_out, meta)
    nc.compile()
    res = run_bass_kernel_spmd(nc, in_maps, core_ids=list(range(NCORES)))
    total = np.zeros(H, np.float64)
    for r in res.results:
        total += r["out"].astype(np.float64).sum(axis=0)
    return (total / node_feats.shape[0]).astype(np.float32)


# revision 36
# speedup vs baseline: 1.5634x; 1.5634x over previous
"""GATv2 8-hop GNN on 8 Trainium2 NeuronCores (Bass/Tile).

Strategy (receiver-sharded, degree-tiered, quad-gather):
  - Nodes are partitioned across the 8 cores balanced by in-degree; each core
    owns its nodes' full in-edge lists (edges sharded by receiver).
  - Per core, nodes are grouped into degree tiers (padded to d in TIERS) and
    packed into tiles of 128 nodes x d edge slots.  Edge data / sender
    indices / masks are laid out tile-major on the host (numpy) so the
    device kernel is a uniform For_i loop per tier.
  - Per hop: each core computes kn|q = h @ [Wk|Wq] for its own nodes (one
    PE matmul per tile from a PE-transposed h), writes kn (f16) to DRAM,
    AllGathers the node-indexed kn table, then processes its tiles:
      * ONE dma_gather per tile fetches, for every edge slot, the 512-byte
        quad-row of the sender (4 nodes' kn packed per row; the table is
        just kn_full viewed as [N/4, 256]).  int16 quad indices fit because
        N/4 < 32768.  Gathers spread over 4 SWDGE queues.
      * a 4-way one-hot select on DVE (mult by host-built sel mask, then
        reduce over the quad axis) extracts each slot's sender row kn[snd].
      * kq = kn[snd] + qb broadcast; per chunk of 8 slots: z = e@We + kq
        (PE matmuls for e@We into PSUM, one DVE add), leaky-relu via
        u = z + (2/3)|z| (logits scaled by 0.6a), exp, mask, and
        mex-weighted reductions on DVE produce den and pooled.
      * h' = relu((pooled/den - qb) * valid)
  - Output: per-core column sums of h; host sums across cores, divides by N.
"""
import os as _os
import sys

sys.path.insert(0, "/opt/trn_rl_repo")

import numpy as np
from contextlib import ExitStack

import concourse.bass as bass
import concourse.mybir as mybir
import concourse.tile as tile
from concourse.bass import AP, ds
from concourse.masks import make_identity

P = 128
H = 64
HOPS = int(_os.environ.get('GAT_HOPS', '8'))
NCORES = 8
CH = 8  # slots per PSUM chunk
NQ = 4  # SWDGE queues for gathers
TIERS = [8, 16, 24, 32, 48, 64, 96, 128]
EPS = 1e-30
F16 = mybir.dt.float16
F32 = mybir.dt.float32
I16 = mybir.dt.int16
U8 = mybir.dt.uint8


def make_bacc(bacc_mod, num_devices=NCORES):
    return bacc_mod.Bacc("TRN2", target_bir_lowering=False, debug=False,
                         num_devices=num_devices, num_swdge_queues=NQ)


# ----------------------------------------------------------------- host prep
def preprocess(node_feats, edge_feats, senders, receivers, Wq, Wk, We, b, att):
    """Shard + tile the graph.  Returns (meta, in_maps)."""
    N = node_feats.shape[0]
    E = edge_feats.shape[0]
    deg = np.bincount(receivers, minlength=N)
    assert deg.max() <= TIERS[-1], f"max degree {deg.max()} > {TIERS[-1]}"

    cum = np.cumsum(deg)
    bounds = [0] + [int(np.searchsorted(cum, E * (c + 1) / NCORES))
                    for c in range(NCORES - 1)] + [N]

    tier_arr = np.asarray(TIERS)
    tier_of = np.searchsorted(tier_arr, np.maximum(deg, 1))
    core_tier_nodes = []
    for c in range(NCORES):
        lo, hi = bounds[c], bounds[c + 1]
        nodes = np.arange(lo, hi)
        t_of = tier_of[lo:hi]
        core_tier_nodes.append([nodes[t_of == ti] for ti in range(len(TIERS))])

    T_tier = [max(int(np.ceil(len(core_tier_nodes[c][ti]) / P))
                  for c in range(NCORES)) for ti in range(len(TIERS))]
    used = [ti for ti in range(len(TIERS)) if T_tier[ti] > 0]
    tiers_d = [TIERS[ti] for ti in used]
    tiers_T = [T_tier[ti] for ti in used]
    Ttot = sum(tiers_T)
    n_slab = Ttot * P
    assert (NCORES * n_slab) % 4 == 0
    assert (NCORES * n_slab) // 4 < 32768, "quad index must fit int16"

    # Global table position: kn_full is laid out [ag_chunk, core, half_slab]
    # so the AllGather can run as two half-table collectives overlapped with
    # phase A.  half must be a multiple of 4 for quad indexing.
    half = (n_slab // 2 + P - 1) // P * P
    assert half % 4 == 0

    def pos_of(c, o):
        ch = o // half
        return ch * NCORES * half + c * half + (o - ch * half)

    pos = np.zeros(N, np.int64)
    slab_node = np.full((NCORES, n_slab), -1, np.int64)
    for c in range(NCORES):
        off = 0
        for j, ti in enumerate(used):
            nodes = core_tier_nodes[c][ti]
            pos[nodes] = pos_of(c, off + np.arange(len(nodes)))
            slab_node[c, off:off + len(nodes)] = nodes
            off += tiers_T[j] * P
    assert off == n_slab

    order = np.argsort(receivers, kind="stable")
    starts = np.zeros(N + 1, np.int64)
    np.cumsum(deg, out=starts[1:])

    ef16 = np.concatenate([np.asarray(edge_feats).astype(np.float16),
                           np.zeros((1, H), np.float16)], axis=0)
    snd_pos = pos[senders]

    in_maps = []
    for c in range(NCORES):
        m = {}
        off = 0
        for j, ti in enumerate(used):
            d = tiers_d[j]
            Tt = tiers_T[j]
            rows = slab_node[c, off:off + Tt * P]
            eids = np.full((Tt * P, d), -1, np.int64)
            real = rows >= 0
            idxr = np.nonzero(real)[0]
            if len(idxr):
                rs = starts[rows[idxr]]
                dd = deg[rows[idxr]]
                ar = np.arange(d)
                sel = ar[None, :] < dd[:, None]
                flat = rs[:, None] + ar[None, :]
                tmp = np.full((len(idxr), d), -1, np.int64)
                tmp[sel] = order[flat[sel]]
                eids[idxr] = tmp
            pad = eids < 0
            e_safe = np.where(pad, E, eids)
            slab = ef16[e_safe].reshape(Tt, P, d, H).transpose(0, 2, 3, 1).copy()
            m[f"edge{j}"] = slab
            spos = np.where(pad, 0, snd_pos[np.maximum(eids, 0)])
            qid = (spos >> 2).astype(np.int16)          # [Tt*P, d]
            phase = (spos & 3).astype(np.int64)
            # int16 wrapped layout for dma_gather: flat i = s*128 + p sits at
            # [i % 16, i // 16], replicated over the 8 gpsimd cores (128 rows).
            qid_t = qid.reshape(Tt, P, d)
            arr = qid_t.transpose(0, 2, 1).reshape(Tt, d * P)   # flat s*128+p
            w16 = arr.reshape(Tt, d * P // 16, 16).transpose(0, 2, 1)
            m[f"idx{j}"] = np.tile(w16, (1, 8, 1)).copy()       # [Tt,128,8d]
            # copy_predicated planes for quad phases 1..3 (phase 0 = base copy)
            mp = np.zeros((Tt * P, d, 3), np.uint8)
            pr = np.nonzero(~pad)
            ph = phase[~pad]
            hit = ph >= 1
            mp[pr[0][hit], pr[1][hit], ph[hit] - 1] = 1.0
            m[f"sel{j}"] = mp.reshape(Tt, P, d, 3).copy()
            m[f"mask{j}"] = (~pad).astype(np.float16).reshape(Tt, P, d).copy()
            off += Tt * P
        rows = slab_node[c]
        h0 = np.zeros((n_slab, H), np.float32)
        rr = rows >= 0
        h0[rr] = np.asarray(node_feats)[rows[rr]].astype(np.float32)
        m["h0"] = h0.reshape(Ttot, P, H).copy()
        m["valid"] = rr.astype(np.float32).reshape(Ttot, P).T.copy()
        wkq = np.concatenate([np.asarray(Wk)[:HOPS], np.asarray(Wq)[:HOPS]],
                             axis=2).astype(np.float16)         # [HOPS,64,128]
        m["wkq"] = wkq
        m["we"] = np.asarray(We)[:HOPS].astype(np.float16)
        m["brep"] = np.broadcast_to(
            np.asarray(b)[:HOPS].astype(np.float16)[:, None, :], (HOPS, P, H)).copy()
        m["arep"] = np.broadcast_to(
            np.asarray(att)[:HOPS].astype(np.float16)[:, None, :],
            (HOPS, P, H)).copy()
        in_maps.append(m)

    meta = dict(tiers_d=tiers_d, tiers_T=tiers_T, Ttot=Ttot, n_slab=n_slab,
                slab_node=slab_node, half=half)
    return meta, in_maps


# -------------------------------------------------------------- device build
def build_program(tc, t_in, t_out, meta):
    """Emit the program into TileContext tc.  t_in: dict name->AP (DRAM)."""
    nc = tc.nc
    tiers_d = meta["tiers_d"]
    tiers_T = meta["tiers_T"]
    Ttot = meta["Ttot"]
    n_slab = meta["n_slab"]

    kn_own = nc.dram_tensor("kn_own", [n_slab, H], F16, kind="Internal")
    kn_full = nc.dram_tensor("kn_full", [NCORES * n_slab, H], F16,
                             kind="Internal", addr_space="Shared")
    kn_own_t = kn_own.ap().rearrange("(t p) f -> t p f", p=P)
    kn_quad = kn_full.ap().rearrange("(q x) f -> q (x f)", x=4)

    SKIP_A = _os.environ.get("GAT_SKIP_A")
    SKIP_AG = _os.environ.get("GAT_SKIP_AG")
    SKIP_C = _os.environ.get("GAT_SKIP_C")
    SKIP_GATHER = _os.environ.get("GAT_SKIP_GATHER")

    with ExitStack() as ctx:
        const = ctx.enter_context(tc.tile_pool(name="const", bufs=1))

        id32 = const.tile([P, P], F32)
        make_identity(nc, id32[:])
        id16 = const.tile([P, P], F16)
        make_identity(nc, id16[:])

        wkq_sb = const.tile([H, HOPS, 2 * H], F16)
        nc.sync.dma_start(wkq_sb[:], t_in["wkq"].rearrange("i k f -> k i f"))
        we_sb = const.tile([H, HOPS, H], F16)
        nc.sync.dma_start(we_sb[:], t_in["we"].rearrange("i k f -> k i f"))
        brep_sb = const.tile([P, HOPS, H], F16)
        nc.sync.dma_start(brep_sb[:], t_in["brep"].rearrange("i p f -> p i f"))
        arep_sb = const.tile([P, HOPS, H], F16)
        nc.sync.dma_start(arep_sb[:], t_in["arep"].rearrange("i p f -> p i f"))
        valid_sb = const.tile([P, Ttot], F32)
        nc.sync.dma_start(valid_sb[:], t_in["valid"])

        h_own = const.tile([P, Ttot, H], F32)
        nc.sync.dma_start(h_own[:], t_in["h0"].rearrange("t p f -> p t f"))
        qb_own = const.tile([P, Ttot, H], F16)

        # hop-invariant per-tile constants kept resident in SBUF
        sel_c = {}
        mask_c = {}
        idx_c = {}
        for j, d in enumerate(tiers_d):
            Tt = tiers_T[j]
            sel_cj = const.tile([P, Tt, d, 3], U8, tag=f"sel_c{j}")
            nc.sync.dma_start(sel_cj[:], t_in[f"sel{j}"].rearrange(
                "t p d q -> p t d q"))
            mask_cj = const.tile([P, Tt, d], F16, tag=f"mask_c{j}")
            nc.sync.dma_start(mask_cj[:], t_in[f"mask{j}"].rearrange(
                "t p d -> p t d"))
            idx_cj = const.tile([P, Tt, 8 * d], I16, tag=f"idx_c{j}")
            nc.sync.dma_start(idx_cj[:], t_in[f"idx{j}"].rearrange(
                "t p c -> p t c"))
            sel_c[j] = sel_cj
            mask_c[j] = mask_cj
            idx_c[j] = idx_cj

        half = meta["half"]
        T_half = half // P

        def emit_ag(lo_rows, hi_rows, out_lo):
            nc.gpsimd.collective_compute(
                "AllGather", mybir.AluOpType.bypass,
                replica_groups=[list(range(NCORES))],
                ins=[kn_own.ap()[lo_rows:hi_rows]],
                outs=[kn_full.ap()[out_lo:out_lo
                                   + NCORES * (hi_rows - lo_rows)]])

        for hop in range(HOPS):
            # ---- phase A: kn table (DRAM) + qb (SBUF) for own nodes.
            # Fully unrolled; the AllGather runs as two half-table chunks,
            # the first overlapping with the second half of phase A.
            with tc.tile_pool(name=f"psA{hop}", bufs=4, space="PSUM") as psA, \
                 tc.tile_pool(name=f"sbA{hop}", bufs=8) as sbA:
                def bodyA(t):
                    h_t = h_own[:, ds(t, 1), :].rearrange("p o f -> p (o f)")
                    h_stage = sbA.tile([P, H], F32, tag="h_stage")
                    nc.vector.tensor_copy(h_stage[:], h_t)
                    trp = psA.tile([H, P], F32, tag="trp")
                    nc.tensor.transpose(trp[:], h_stage[:], id32[:])
                    hT16 = sbA.tile([H, P], F16, tag="hT16")
                    nc.vector.tensor_copy(hT16[:], trp[:])
                    knq = psA.tile([P, 2 * H], F32, tag="knq")
                    nc.tensor.matmul(knq[:], lhsT=hT16[:],
                                     rhs=wkq_sb[:, hop, :], start=True, stop=True)
                    kn16 = sbA.tile([P, H], F16, tag="kn16")
                    nc.scalar.copy(kn16[:], knq[:, 0:H])
                    nc.sync.dma_start(
                        kn_own_t[ds(t, 1)].rearrange("o p f -> p (o f)"), kn16[:])
                    qb_t = qb_own[:, ds(t, 1), :].rearrange("p o f -> p (o f)")
                    nc.vector.tensor_tensor(out=qb_t, in0=knq[:, H:2 * H],
                                            in1=brep_sb[:, hop, :],
                                            op=mybir.AluOpType.add)

                if not SKIP_A:
                    for t in range(T_half):
                        bodyA(t)
                    if not SKIP_AG:
                        emit_ag(0, half, 0)
                    for t in range(T_half, Ttot):
                        bodyA(t)
                    if not SKIP_AG:
                        emit_ag(half, n_slab, NCORES * half)
                else:
                    nc.vector.memset(qb_own[:], 0)
                    if not SKIP_AG:
                        emit_ag(0, half, 0)
                        emit_ag(half, n_slab, NCORES * half)

            # ---- phase C: edge processing, one For_i per tier
            tile_base = 0
            for j, d in enumerate(tiers_d):
                Tt = tiers_T[j]
                nchunk = (d + CH - 1) // CH
                qb_tier = qb_own[:, tile_base:tile_base + Tt, :]
                h_tier = h_own[:, tile_base:tile_base + Tt, :]
                valid_tier = valid_sb[:, tile_base:tile_base + Tt]
                gbufs = 4 if d <= 24 else 2
                with tc.tile_pool(name=f"psC{hop}_{j}", bufs=4, space="PSUM") as psC, \
                     tc.tile_pool(name=f"psP{hop}_{j}", bufs=4, space="PSUM") as psP, \
                     tc.tile_pool(name=f"gth{hop}_{j}", bufs=gbufs) as gth, \
                     tc.tile_pool(name=f"sbC{hop}_{j}", bufs=4 if d <= 24 else 2) as sbC:
                    def bodyC(t, qnum, d=d, j=j, Tt=Tt, nchunk=nchunk,
                              qb_tier=qb_tier, h_tier=h_tier,
                              valid_tier=valid_tier, psC=psC, psP=psP,
                              gth=gth, sbC=sbC):
                        knr4 = gth.tile([P, d, 4 * H], F16, tag="knr4")
                        if SKIP_GATHER:
                            nc.vector.memset(knr4[:], 0)
                        else:
                            nc.gpsimd.dma_gather(
                                out_ap=knr4[:, :, :],
                                in_ap=kn_quad,
                                idxs_ap=idx_c[j][:, ds(t, 1), :].rearrange(
                                    "p o c -> p (o c)"),
                                num_idxs=d * P,
                                num_idxs_reg=d * P,
                                elem_size=4 * H,
                                single_packet=False,
                                queue_num=qnum)
        # same math as reference: leaky_relu(z, 0.2) = max(0.2*z, z)
                        qb_t = qb_tier[:, ds(t, 1), :].rearrange("p o f -> p (o f)")
                        mask_t = mask_c[j][:, ds(t, 1), :].rearrange(
                            "p o d -> p (o d)")
                        z16all = sbC.tile([P, d, H], F16, tag="z16all")
                        lgall = sbC.tile([P, d], F32, tag="lgall")
                        mexall = sbC.tile([P, d], F16, tag="mexall")
                        pooled_ps = psP.tile([P, H], F32, tag="pool")
                        for k in range(nchunk):
                            c0 = k * CH
                            cw = min(CH, d - c0)
                            # quad select: copy phase-0 candidate, then
                            # overwrite with phases 1..3 via copy_predicated
                            kq = sbC.tile([P, CH, H], F16, tag="kq")
                            nc.vector.tensor_copy(
                                kq[:, 0:cw, :], knr4[:, c0:c0 + cw, 0:H])
                            for q in range(1, 4):
                                _s = sel_c[j][:, ds(t, 1), c0:c0 + cw,
                                              q - 1:q]
                                mp_b = AP(_s.tensor, _s.offset,
                                          [list(_s.ap[0]), list(_s.ap[2]),
                                           [0, H]])
                                nc.vector.copy_predicated(
                                    kq[:, 0:cw, :], mp_b,
                                    knr4[:, c0:c0 + cw, q * H:(q + 1) * H])
                            edge_sb = sbC.tile([H, CH, P], F16, tag="edge")
                            nc.sync.dma_start(
                                edge_sb[:, 0:cw, :],
                                t_in[f"edge{j}"][ds(t, 1), c0:c0 + cw]
                                .rearrange("o s f p -> f (o s) p"))
                            zps = psC.tile([P, CH * H], F32, tag="z")
                            for s in range(cw):
                                sl = slice(s * H, (s + 1) * H)
                                nc.tensor.matmul(zps[:, sl], lhsT=edge_sb[:, s, :],
                                                 rhs=we_sb[:, hop, :],
                                                 start=True, stop=True)
                            _q = qb_t
                            qb_b = AP(_q.tensor, _q.offset,
                                      [list(_q.ap[0]), [0, cw], list(_q.ap[1])])
                            kqq = sbC.tile([P, CH, H], F16, tag="kqq")
                            nc.vector.tensor_tensor(out=kqq[:, 0:cw, :],
                                                    in0=kq[:, 0:cw, :],
                                                    in1=qb_b,
                                                    op=mybir.AluOpType.add)
                            z16c = z16all[:, c0:c0 + cw, :]
                            nc.vector.tensor_tensor(
                                out=z16c,
                                in0=zps[:, 0:cw * H].rearrange(
                                    "p (c h) -> p c h", h=H),
                                in1=kqq[:, 0:cw, :],
                                op=mybir.AluOpType.add)
                            # leaky_relu(z, 0.2) = max(0.2*z, z); u/au/logits
                            # run on the (otherwise idle) gpsimd vector units
                            u = sbC.tile([P, CH, H], F16, tag="u")
                            nc.vector.scalar_tensor_tensor(
                                out=u[:, 0:cw, :], in0=z16c,
                                scalar=0.2, in1=z16c,
                                op0=mybir.AluOpType.mult, op1=mybir.AluOpType.max)
                            au = sbC.tile([P, CH, H], F16, tag="au")
                            _a = arep_sb[:, hop, :]
                            a_b = AP(_a.tensor, _a.offset,
                                     [list(_a.ap[0]), [0, cw], list(_a.ap[1])])
                            nc.vector.tensor_tensor(
                                out=au[:, 0:cw, :],
                                in0=u[:, 0:cw, :],
                                in1=a_b, op=mybir.AluOpType.mult)
                            nc.vector.tensor_reduce(
                                out=lgall[:, c0:c0 + cw], in_=au[:, 0:cw, :],
                                axis=mybir.AxisListType.X, op=mybir.AluOpType.add)
                        # tile-wide softmax pieces: one Exp per tile keeps the
                        # ACT engine on a single function table
                        ex = sbC.tile([P, d], F32, tag="ex")
                        nc.scalar.activation(ex[:], lgall[:],
                                             mybir.ActivationFunctionType.Exp)
                        nc.vector.tensor_tensor(
                            out=mexall[:], in0=ex[:], in1=mask_t,
                            op=mybir.AluOpType.mult)
                        for k in range(nchunk):
                            c0 = k * CH
                            cw = min(CH, d - c0)
                            exz = sbC.tile([P, CH, H], F16, tag="exz")
                            _m = mexall[:, c0:c0 + cw]
                            m_b = AP(_m.tensor, _m.offset,
                                     [list(_m.ap[0]), list(_m.ap[1]), [0, H]])
                            nc.vector.tensor_tensor(
                                out=exz[:, 0:cw, :],
                                in0=z16all[:, c0:c0 + cw, :],
                                in1=m_b, op=mybir.AluOpType.mult)
                            # pooled += sum_s exz_s on the idle PE via
                            # identity matmuls (avoids slow strided reduces)
                            for s in range(cw):
                                nc.tensor.matmul(
                                    pooled_ps[:], lhsT=id16[:],
                                    rhs=exz[:, s, :],
                                    start=(k == 0 and s == 0),
                                    stop=(k == nchunk - 1 and s == cw - 1))
                        den_r = sbC.tile([P, 1], F32, tag="den_r")
                        nc.vector.tensor_reduce(
                            out=den_r[:], in_=mexall[:],
                            axis=mybir.AxisListType.X, op=mybir.AluOpType.add)
                        den = sbC.tile([P, 1], F32, tag="den")
                        nc.vector.tensor_scalar(
                            out=den[:], in0=den_r[:], scalar1=EPS,
                            scalar2=None, op0=mybir.AluOpType.add)
                        rden = sbC.tile([P, 1], F32, tag="rden")
                        nc.vector.reciprocal(rden[:], den[:])
                        t1 = sbC.tile([P, H], F32, tag="t1")
                        nc.vector.scalar_tensor_tensor(
                            out=t1[:], in0=pooled_ps[:], scalar=rden[:],
                            in1=qb_t,
                            op0=mybir.AluOpType.mult,
                            op1=mybir.AluOpType.subtract)
                        h_t = h_tier[:, ds(t, 1), :].rearrange("p o f -> p (o f)")
                        nc.vector.tensor_scalar(
                            out=h_t, in0=t1[:], scalar1=valid_tier[:, ds(t, 1)],
                            scalar2=0.0, op0=mybir.AluOpType.mult,
                            op1=mybir.AluOpType.max)

                    if not SKIP_C:
                        for t in range(Tt):
                            bodyC(t, (tile_base + t) % NQ)
                tile_base += Tt

        hsum = const.tile([P, H], F32)
        nc.vector.tensor_reduce(out=hsum[:],
                                in_=h_own[:].rearrange("p t f -> p f t"),
                                axis=mybir.AxisListType.X, op=mybir.AluOpType.add)
        nc.sync.dma_start(t_out, hsum[:])


# --------------------------------------------------------------------- entry
def kernel(node_feats, edge_feats, senders, receivers, Wq, Wk, We, b, att):
    from concourse import bacc
    from concourse.bass_utils import run_bass_kernel_spmd

    node_feats = np.asarray(node_feats)
    meta, in_maps = preprocess(node_feats, edge_feats, senders, receivers,
                               Wq, Wk, We, b, att)
    nc = make_bacc(bacc)
    t_in = {}
    for k, v in in_maps[0].items():
        t_in[k] = nc.dram_tensor(k, list(v.shape), mybir.dt.from_np(v.dtype),
                                 kind="ExternalInput").ap()
    t_out = nc.dram_tensor("out", [P, H], F32, kind="ExternalOutput").ap()
    with tile.TileContext(nc) as tc:
        build_program(tc, t_in, t_out, meta)
    nc.compile()
    res = run_bass_kernel_spmd(nc, in_maps, core_ids=list(range(NCORES)))
    total = np.zeros(H, np.float64)
    for r in res.results:
        total += r["out"].astype(np.float64).sum(axis=0)
    return (total / node_feats.shape[0]).astype(np.float32)
